# revision 76
# baseline (speedup 1.0000x reference)
"""Trainium2 Bass kernel for DensePairwiseRelaxedWordMoverSimilarity.

Shapes (hardcoded): x1 (64,128,512) f32, mask1 (64,128) bool,
                    x2 (64,128,512) f32, mask2 (64,128) bool -> out (64,64) f32.

Strategy: data-parallel over B1 across 8 cores; core k holds x1 rows
[8k, 8k+8) plus the full x2 and produces an (8, 64) output slab.

v3: single-orientation fp8 matmul + exp-domain reductions.
For each (a, 4-b chunk) the PE computes C^T tiles [128 s, 4b x 128 t]
(fp8 DoubleRow, f32 PSUM).  The scalar engine applies
E = exp(beta*(C - c0)) into bf16 SBUF (trimmed to the chunk's baked
t-extent).  Then BOTH reductions come cheap:
  - sim1[a,b,s] = max_t C = (ln max_t E)/beta + c0: segmented DVE
    reduce_max over the free dim (optionally gpsimd pre-halving),
  - sim2[a,b,t] via log-sum-exp: sum_s E is a PE matmul with an
    all-ones stationary column, accumulated per (chunk, a) into one
    PSUM bank row; masked-s rows contribute exp(-beta*c0) ~ 1e-38 (b
    columns are sorted+trimmed so masked-t never pollutes sums).
    The LSE overshoot at beta=250 measures ~3e-3 rel on this data,
    under the fp8 quantization floor.
One Ln pass per collection (+1e-36 bias so memset-zero pad columns
stay finite) and tiny w1/w2-weighted matmul/reduce means finish on
device; host adds the (m1+m2)/beta + c0 affine and unsorts b.
"""

import numpy as np
import ml_dtypes

import concourse.bacc as bacc
import concourse.mybir as mybir
from concourse import tile
from concourse.bass_utils import run_bass_kernel_spmd

F32 = mybir.dt.float32
BF16 = mybir.dt.bfloat16
FP8 = mybir.dt.float8e4
AX = mybir.AxisListType
AF = mybir.ActivationFunctionType
ALU = mybir.AluOpType
DR = mybir.MatmulPerfMode.DoubleRow

E4NP = ml_dtypes.float8_e4m3
BFNP = ml_dtypes.bfloat16

B1, S1, B2, S2, D = 64, 128, 64, 128, 512
NCORES = 8
A = B1 // NCORES          # 8 x1 rows per core
X1W = A * S1              # 1024 x1 columns
X2W = B2 * S2             # 8192 x2 columns
NCH = 16                  # chunks of 4 sorted b's (512 x2 columns)
QSCALE = 32.0             # fp8 pre-scale; sims carry QSCALE^2
BETA = 250.0              # LSE sharpness
C0 = 0.35                 # exp recentering; keeps E <= ~1
LNEPS = 1e-36             # (unused since bit-log; kept in consts layout)
DEFER = 6                 # units of lag for E consumers (PE slack)

_CACHE = {}


def _build(L2G):
    """L2G: 16 baked t-extents (mult of 4), one per sorted 4-b chunk.
    x2 arrives host-packed: chunk c's 4 b-columns sit back-to-back at
    stride L2G[c] starting at byte offset OFF[c], so the DR matmuls
    stream only ~78% of the columns at no extra instruction cost."""
    OFF = [0]
    for c in range(NCH):
        OFF.append(OFF[-1] + 4 * L2G[c])
    W = OFF[NCH]
    nc = bacc.Bacc(None, target_bir_lowering=False, debug=False)

    x1p = nc.declare_dram_parameter("x1p", [D, X1W], FP8, isOutput=False)
    x2p = nc.declare_dram_parameter("x2p", [D, W], FP8, isOutput=False)
    consts = nc.declare_dram_parameter("consts", [128, 4], F32, isOutput=False)
    maxEo = nc.declare_dram_parameter("maxEo", [128, A * B2], BF16, isOutput=True)
    sumEo = nc.declare_dram_parameter("sumEo", [128, 512], F32, isOutput=True)

    exp_scale = float(BETA / (QSCALE * QSCALE))

    with tile.TileContext(nc) as tc:
        with (
            tc.tile_pool(name="xts", bufs=1) as xts_pool,
            tc.tile_pool(name="cst", bufs=1) as cst_pool,
            tc.tile_pool(name="coll", bufs=1) as coll_pool,
            tc.tile_pool(name="epool", bufs=8) as epool,
            tc.tile_pool(name="hpool", bufs=3) as hpool,
            tc.tile_pool(name="psS", bufs=3, space="PSUM") as psS,
            tc.tile_pool(name="psF", bufs=1, space="PSUM") as psF,
        ):
            # ---- loads: first chunk + x1 first (they gate the first
            # matmul), then constants, then the remaining x2 blocks.
            x2c = [None] * NCH
            blocks = [(0, 1), (1, 1), (2, 2), (4, 4), (8, 8)]
            xb0 = xts_pool.tile([128, 4, 4 * L2G[0]], FP8, tag="xb0")
            nc.sync.dma_start(
                xb0[:],
                x2p.ap()[:, OFF[0] : OFF[1]].rearrange(
                    "(k p) m -> p k m", p=128
                ),
            )
            x2c[0] = xb0[:, :, :]
            # x1 lands as four a-pair tiles so unit 0 can start after the
            # first one (tile-granular dependency tracking)
            x1q = []
            for q in range(4):
                xq = xts_pool.tile([128, 4, 256], FP8, tag=f"x1q{q}")
                nc.sync.dma_start(
                    xq[:],
                    x1p.ap()[:, q * 256 : (q + 1) * 256].rearrange(
                        "(k p) m -> p k m", p=128
                    ),
                )
                x1q.append(xq)
                if q == 0:
                    csts = cst_pool.tile([128, 4], F32, tag="consts")
                    nc.sync.dma_start(csts[:], consts.ap())
            for g0, w in blocks[1:]:
                wcols = OFF[g0 + w] - OFF[g0]
                xb = xts_pool.tile([128, 4, wcols], FP8, tag=f"xb{g0}")
                nc.sync.dma_start(
                    xb[:],
                    x2p.ap()[:, OFF[g0] : OFF[g0 + w]].rearrange(
                        "(k p) m -> p k m", p=128
                    ),
                )
                for j in range(w):
                    o0 = OFF[g0 + j] - OFF[g0]
                    o1 = OFF[g0 + j + 1] - OFF[g0]
                    x2c[g0 + j] = xb[:, :, o0:o1]

            expbias = csts[:, 1:2]      # -BETA*C0
            # sliding-window one-hot: col 64 is all-ones, so the width-64
            # slice [64-m : 128-m] has its ones in column m.
            oh64 = cst_pool.tile([128, 128], BF16, tag="oh64")
            nc.vector.memset(oh64[:], 0.0)
            nc.vector.tensor_copy(
                oh64[:, 64:65], csts[:, 0:1]
            )

            # maxE collection [s, a, sorted-b]; SumE rows (8c+a) % 64 live
            # in a per-chunk-half PSUM bank so the finished half can drain
            # while the PE still accumulates the other (PSUM bank W/R
            # collisions are fatal; separate banks dodge the serialization)
            sim1st = coll_pool.tile([128, A, B2], BF16, tag="sim1st")
            sumE = []
            for h in range(2):
                sE = psF.tile([64, 512], F32, tag=f"sumE{h}")
                nc.vector.memset(sE[:], 0.0)
                sumE.append(sE)

            def emit_mm(u):
                """fp8 DoubleRow matmuls for unit u = (chunk, a-pair);
                the host-packed moving streams only valid columns."""
                c, ap_ = u // 4, (u % 4) * 2
                w = 4 * L2G[c]
                S = psS.tile([128, 2, 512], F32, tag="S", name=f"S{u}")
                for j in range(2):
                    a = ap_ + j
                    for i, (k0, k1) in enumerate(((0, 2), (2, 4))):
                        nc.tensor.matmul(
                            S[:, j, 0:w],
                            x1q[a // 2][:, k0:k1, (a % 2) * 128 : (a % 2) * 128 + 128],
                            x2c[c][:, k0:k1, :],
                            start=(i == 0),
                            stop=(i == 1),
                            perf_mode=DR,
                        )
                return S

            def emit_exp(u, S):
                """ACT: E = exp(scale*C - beta*c0) over the packed cols."""
                c = u // 4
                w = 4 * L2G[c]
                E = epool.tile([128, 2, 512], BF16, tag="E", name=f"E{u}")
                nc.scalar.activation(
                    E[:, :, 0:w], S[:, :, 0:w], AF.Exp,
                    bias=expbias, scale=exp_scale,
                )
                return E

            def emit_sum(u, E):
                """PE: SumE row 8c+a += onehot64^T @ E.  Rows live in two
                64-row halves (legal matmul base partitions 0/64); each
                half is one PSUM accumulation chain over its 64 writes.
                The strided output AP re-spreads the packed columns onto
                uniform 128-stride segments."""
                c, ap_ = u // 4, (u % 4) * 2
                l = L2G[c]
                for j in range(2):
                    a = ap_ + j
                    r = 8 * c + a
                    h, m = r // 64, r % 64
                    sv = sumE[h][:].rearrange("p (g t) -> p g t", g=4)
                    nc.tensor.matmul(
                        sv[:, :, 0:l],
                        oh64[:, 64 - m : 128 - m],
                        E[:, j, 0 : 4 * l],
                        start=(m == 0),
                        stop=(m == 63),
                    )

            def emit_red(u, E):
                """sim1 maxE: one tensor_tensor max level (bf16 2x_1p)
                halves the elements the 1x-only reduce_max must stream."""
                c, ap_ = u // 4, (u % 4) * 2
                l = L2G[c]
                dst = sim1st[:, ap_ : ap_ + 2, 4 * c : 4 * c + 4]
                h = l // 2
                Ev = E[:, :, 0 : 4 * l].rearrange("p j (g t) -> p j g t", g=4)
                H = hpool.tile([128, 2, 4, 64], BF16, tag="H", name=f"H{u}")
                nc.vector.tensor_max(
                    H[:, :, :, 0:h], Ev[:, :, :, 0:h], Ev[:, :, :, h:l]
                )
                # contiguous H lets the reduce present a 3D stride-uniform
                # AP (4D strided inputs fall off the 2x_1p fast path)
                Hv = H[:].rearrange("p j g t -> p (j g) t")
                nc.vector.reduce_max(dst, Hv[:, :, 0:h], axis=AX.X)

            # ---- finale: ship the small collections; the host takes the
            # exact logs and w-weighted means (O(B1*B2*(S1+S2)) scalar
            # work, ~0.003% of the device FLOPs).
            lnS = coll_pool.tile([128, 512], F32, tag="lnS")

            def emit_finale_m2(h):
                """PSUM->SBUF copy + DMA for the h-th half-bank of SumE
                (needs only that half's Sigma chain, not the reduces)."""
                p0, p1 = 64 * h, 64 * (h + 1)
                nc.vector.tensor_copy(lnS[p0:p1, :], sumE[h][:])
                nc.sync.dma_start(sumEo.ap()[p0:p1, :], lnS[p0:p1, :])

            # software pipeline: PE mms for unit u, then E-consumers for
            # two lagged units at once (fewer DR<->onehot stationary
            # transitions on the PE).
            pending = []

            def drain_pending(n):
                while pending and len(pending) > n:
                    pu, pE = pending.pop(0)
                    emit_sum(pu, pE)
                    emit_red(pu, pE)
                    if pu == 31:
                        emit_finale_m2(0)

            for u in range(NCH * 4):
                S = emit_mm(u)
                E = emit_exp(u, S)
                pending.append((u, E))
                if u % 2 and len(pending) > DEFER:
                    drain_pending(DEFER - 1)
            # final drain: all Sigma-mms first so m2's half-1 chain closes
            # early, then the remaining maxE reduces, then m1.
            for pu, pE in pending:
                emit_sum(pu, pE)
            emit_finale_m2(1)
            for pu, pE in pending:
                emit_red(pu, pE)
            pending = []
            nc.sync.dma_start(maxEo.ap(), sim1st[:])
    nc.finalize()
    return nc


def _prep(x1, mask1, x2, mask2):
    """Host-side marshaling: normalize, mask-zero, sort b, quantize."""
    x1 = np.asarray(x1, dtype=np.float32)
    x2 = np.asarray(x2, dtype=np.float32)
    m1 = np.asarray(mask1).astype(bool)
    m2 = np.asarray(mask2).astype(bool)

    EPS = 1e-8
    n1 = np.sqrt((x1 * x1).sum(-1, keepdims=True))
    n2 = np.sqrt((x2 * x2).sum(-1, keepdims=True))
    x1n = (x1 / np.maximum(n1, EPS)) * QSCALE
    x2n = (x2 / np.maximum(n2, EPS)) * QSCALE
    x1n[~m1] = 0.0
    x2n[~m2] = 0.0

    len1 = m1.sum(axis=1).astype(np.int64)
    len2 = m2.sum(axis=1).astype(np.int64)
    ext2 = np.where(m2.any(1), S2 - np.argmax(m2[:, ::-1], axis=1), 1)
    # ascending: the first chunks are the cheapest, so the pipeline fills
    # while the input DMA stream is still catching up
    b_order = np.argsort(ext2, kind="stable")

    def _ev(v):
        v = int(max(v, 4))
        return (v + 3) // 4 * 4   # /2-able and 4B-aligned halves (2x_1p)

    L2G = tuple(_ev(ext2[b_order[4 * c + 3]]) for c in range(NCH))

    w1 = m1.astype(np.float32) * (0.5 / np.maximum(len1, 1))[:, None]
    w2 = m2.astype(np.float32) * (0.5 / np.maximum(len2, 1))[:, None]
    w2s = w2[b_order]                                 # [64 sorted b, 128 t]

    # pack x2 columns: chunk c's 4 b's back-to-back at stride L2G[c]
    x2s = x2n[b_order]                                # [64, 128, 512]
    Wp = 4 * sum(L2G)
    x2pk = np.zeros((Wp, D), np.float32)
    off = 0
    for c in range(NCH):
        l = L2G[c]
        for g in range(4):
            x2pk[off : off + min(l, S2)] = x2s[4 * c + g, :l]
            off += l
    x2T = np.ascontiguousarray(x2pk.T).astype(E4NP)

    in_maps = []
    for k in range(NCORES):
        rows = slice(k * A, (k + 1) * A)
        x1T = np.ascontiguousarray(
            x1n[rows].reshape(X1W, D).T
        ).astype(E4NP)
        consts = np.zeros((128, 4), np.float32)
        consts[:, 0] = 1.0
        consts[:, 1] = -BETA * C0
        in_maps.append(
            {
                "x1p": x1T,
                "x2p": x2T,
                "consts": consts,
            }
        )
    return in_maps, b_order, L2G, (w1, w2s)


def kernel(x1, mask1, x2, mask2):
    in_maps, b_order, key, (w1, w2s) = _prep(x1, mask1, x2, mask2)
    if _CACHE.get("key") != key:
        _CACHE["nc"] = _build(key)
        _CACHE["key"] = key
    nc = _CACHE["nc"]
    res = run_bass_kernel_spmd(nc, in_maps, list(range(NCORES)))
    # host finale: exact logs + w-weighted means of the shipped
    # collections (both w tables already carry the 1/2 factor)
    w2v = w2s.reshape(NCH, 4, S2)                     # [c, g, t]
    outp = np.zeros((B1, B2), dtype=np.float32)
    for k in range(NCORES):
        maxE = res.results[k]["maxEo"].reshape(128, A, B2).astype(np.float32)
        sumE = res.results[k]["sumEo"]                # [128 (c,a), 512]
        lnm = np.log(np.maximum(maxE, 1e-38))         # [s, a, sorted b]
        m1v = np.einsum("sab,as->ab", lnm, w1[k * A : (k + 1) * A])
        lnS = np.log(np.maximum(sumE, 1e-45)).reshape(NCH, A, 4, S2)
        M2 = np.einsum("cagt,cgt->acg", lnS, w2v).reshape(A, B2)
        vals = (m1v + M2) / BETA + C0
        outp[np.ix_(range(k * A, (k + 1) * A), b_order)] = vals
    return np.ascontiguousarray(outp)


# revision 77
# speedup vs baseline: 1.0099x; 1.0099x over previous
"""Trainium2 Bass kernel for DensePairwiseRelaxedWordMoverSimilarity.

Shapes (hardcoded): x1 (64,128,512) f32, mask1 (64,128) bool,
                    x2 (64,128,512) f32, mask2 (64,128) bool -> out (64,64) f32.

Strategy: data-parallel over B1 across 8 cores; core k holds x1 rows
[8k, 8k+8) plus the full x2 and produces an (8, 64) output slab.

v3: single-orientation fp8 matmul + exp-domain reductions.
For each (a, 4-b chunk) the PE computes C^T tiles [128 s, 4b x 128 t]
(fp8 DoubleRow, f32 PSUM).  The scalar engine applies
E = exp(beta*(C - c0)) into bf16 SBUF (trimmed to the chunk's baked
t-extent).  Then BOTH reductions come cheap:
  - sim1[a,b,s] = max_t C = (ln max_t E)/beta + c0: segmented DVE
    reduce_max over the free dim (optionally gpsimd pre-halving),
  - sim2[a,b,t] via log-sum-exp: sum_s E is a PE matmul with an
    all-ones stationary column, accumulated per (chunk, a) into one
    PSUM bank row; masked-s rows contribute exp(-beta*c0) ~ 1e-38 (b
    columns are sorted+trimmed so masked-t never pollutes sums).
    The LSE overshoot at beta=250 measures ~3e-3 rel on this data,
    under the fp8 quantization floor.
One Ln pass per collection (+1e-36 bias so memset-zero pad columns
stay finite) and tiny w1/w2-weighted matmul/reduce means finish on
device; host adds the (m1+m2)/beta + c0 affine and unsorts b.
"""

import numpy as np
import ml_dtypes

import concourse.bacc as bacc
import concourse.mybir as mybir
from concourse import tile
from concourse.bass_utils import run_bass_kernel_spmd

F32 = mybir.dt.float32
BF16 = mybir.dt.bfloat16
FP8 = mybir.dt.float8e4
AX = mybir.AxisListType
AF = mybir.ActivationFunctionType
ALU = mybir.AluOpType
DR = mybir.MatmulPerfMode.DoubleRow

E4NP = ml_dtypes.float8_e4m3
BFNP = ml_dtypes.bfloat16

B1, S1, B2, S2, D = 64, 128, 64, 128, 512
NCORES = 8
A = B1 // NCORES          # 8 x1 rows per core
X1W = A * S1              # 1024 x1 columns
X2W = B2 * S2             # 8192 x2 columns
NCH = 16                  # chunks of 4 sorted b's (512 x2 columns)
QSCALE = 32.0             # fp8 pre-scale; sims carry QSCALE^2
BETA = 250.0              # LSE sharpness
C0 = 0.35                 # exp recentering; keeps E <= ~1
LNEPS = 1e-36             # (unused since bit-log; kept in consts layout)
DEFER = 6                 # units of lag for E consumers (PE slack)

_CACHE = {}


def _build(L2G):
    """L2G: 16 baked t-extents (mult of 4), one per sorted 4-b chunk.
    x2 arrives host-packed: chunk c's 4 b-columns sit back-to-back at
    stride L2G[c] starting at byte offset OFF[c], so the DR matmuls
    stream only ~78% of the columns at no extra instruction cost."""
    OFF = [0]
    for c in range(NCH):
        OFF.append(OFF[-1] + 4 * L2G[c])
    W = OFF[NCH]
    nc = bacc.Bacc(None, target_bir_lowering=False, debug=False)

    x1p = nc.declare_dram_parameter("x1p", [D, X1W], FP8, isOutput=False)
    x2p = nc.declare_dram_parameter("x2p", [D, W], FP8, isOutput=False)
    consts = nc.declare_dram_parameter("consts", [128, 4], F32, isOutput=False)
    maxEo = nc.declare_dram_parameter("maxEo", [128, A * B2], BF16, isOutput=True)
    sumEo = nc.declare_dram_parameter("sumEo", [128, 512], F32, isOutput=True)

    exp_scale = float(BETA / (QSCALE * QSCALE))

    with tile.TileContext(nc) as tc:
        with (
            tc.tile_pool(name="xts", bufs=1) as xts_pool,
            tc.tile_pool(name="cst", bufs=1) as cst_pool,
            tc.tile_pool(name="coll", bufs=1) as coll_pool,
            tc.tile_pool(name="epool", bufs=8) as epool,
            tc.tile_pool(name="hpool", bufs=3) as hpool,
            tc.tile_pool(name="psS", bufs=3, space="PSUM") as psS,
            tc.tile_pool(name="psF", bufs=1, space="PSUM") as psF,
        ):
            # ---- loads: first chunk + x1 first (they gate the first
            # matmul), then constants, then the remaining x2 blocks.
            x2c = [None] * NCH
            blocks = [(0, 1), (1, 1), (2, 2), (4, 4), (8, 8)]
            xb0 = xts_pool.tile([128, 4, 4 * L2G[0]], FP8, tag="xb0")
            nc.sync.dma_start(
                xb0[:],
                x2p.ap()[:, OFF[0] : OFF[1]].rearrange(
                    "(k p) m -> p k m", p=128
                ),
            )
            x2c[0] = xb0[:, :, :]
            # x1 lands as four a-pair tiles so unit 0 can start after the
            # first one (tile-granular dependency tracking)
            x1q = []
            for q in range(4):
                xq = xts_pool.tile([128, 4, 256], FP8, tag=f"x1q{q}")
                nc.sync.dma_start(
                    xq[:],
                    x1p.ap()[:, q * 256 : (q + 1) * 256].rearrange(
                        "(k p) m -> p k m", p=128
                    ),
                )
                x1q.append(xq)
                if q == 0:
                    csts = cst_pool.tile([128, 4], F32, tag="consts")
                    nc.sync.dma_start(csts[:], consts.ap())
            for g0, w in blocks[1:]:
                wcols = OFF[g0 + w] - OFF[g0]
                xb = xts_pool.tile([128, 4, wcols], FP8, tag=f"xb{g0}")
                nc.sync.dma_start(
                    xb[:],
                    x2p.ap()[:, OFF[g0] : OFF[g0 + w]].rearrange(
                        "(k p) m -> p k m", p=128
                    ),
                )
                for j in range(w):
                    o0 = OFF[g0 + j] - OFF[g0]
                    o1 = OFF[g0 + j + 1] - OFF[g0]
                    x2c[g0 + j] = xb[:, :, o0:o1]

            expbias = csts[:, 1:2]      # -BETA*C0
            # sliding-window one-hot: col 64 is all-ones, so the width-64
            # slice [64-m : 128-m] has its ones in column m.
            oh64 = cst_pool.tile([128, 128], BF16, tag="oh64")
            nc.vector.memset(oh64[:], 0.0)
            nc.vector.tensor_copy(
                oh64[:, 64:65], csts[:, 0:1]
            )

            # maxE collection [s, a, sorted-b]; SumE rows (8c+a) % 64 live
            # in a per-chunk-half PSUM bank so the finished half can drain
            # while the PE still accumulates the other (PSUM bank W/R
            # collisions are fatal; separate banks dodge the serialization)
            sim1st = coll_pool.tile([128, A, B2], BF16, tag="sim1st")
            sumE = []
            for h in range(2):
                sE = psF.tile([64, 512], F32, tag=f"sumE{h}")
                nc.vector.memset(sE[:], 0.0)
                sumE.append(sE)

            def emit_mm(u):
                """fp8 DoubleRow matmuls for unit u = (chunk, a-pair);
                the host-packed moving streams only valid columns."""
                c, ap_ = u // 4, (u % 4) * 2
                w = 4 * L2G[c]
                S = psS.tile([128, 2, 512], F32, tag="S", name=f"S{u}")
                for j in range(2):
                    a = ap_ + j
                    for i, (k0, k1) in enumerate(((0, 2), (2, 4))):
                        nc.tensor.matmul(
                            S[:, j, 0:w],
                            x1q[a // 2][:, k0:k1, (a % 2) * 128 : (a % 2) * 128 + 128],
                            x2c[c][:, k0:k1, :],
                            start=(i == 0),
                            stop=(i == 1),
                            perf_mode=DR,
                        )
                return S

            def emit_exp(u, S):
                """ACT: E = exp(scale*C - beta*c0) over the packed cols."""
                c = u // 4
                w = 4 * L2G[c]
                E = epool.tile([128, 2, 512], BF16, tag="E", name=f"E{u}")
                nc.scalar.activation(
                    E[:, :, 0:w], S[:, :, 0:w], AF.Exp,
                    bias=expbias, scale=exp_scale,
                )
                return E

            def emit_sum(u, E):
                """PE: SumE row 8c+a += onehot64^T @ E.  Rows live in two
                64-row halves (legal matmul base partitions 0/64); each
                half is one PSUM accumulation chain over its 64 writes.
                The strided output AP re-spreads the packed columns onto
                uniform 128-stride segments."""
                c, ap_ = u // 4, (u % 4) * 2
                l = L2G[c]
                for j in range(2):
                    a = ap_ + j
                    r = 8 * c + a
                    h, m = r // 64, r % 64
                    sv = sumE[h][:].rearrange("p (g t) -> p g t", g=4)
                    nc.tensor.matmul(
                        sv[:, :, 0:l],
                        oh64[:, 64 - m : 128 - m],
                        E[:, j, 0 : 4 * l],
                        start=(m == 0),
                        stop=(m == 63),
                    )

            def emit_red(u, E):
                """sim1 maxE: one tensor_tensor max level (bf16 2x_1p)
                halves the elements the 1x-only reduce_max must stream."""
                c, ap_ = u // 4, (u % 4) * 2
                l = L2G[c]
                dst = sim1st[:, ap_ : ap_ + 2, 4 * c : 4 * c + 4]
                h = l // 2
                Ev = E[:, :, 0 : 4 * l].rearrange("p j (g t) -> p j g t", g=4)
                H = hpool.tile([128, 2, 4, 64], BF16, tag="H", name=f"H{u}")
                nc.vector.tensor_max(
                    H[:, :, :, 0:h], Ev[:, :, :, 0:h], Ev[:, :, :, h:l]
                )
                # contiguous H lets the reduce present a 3D stride-uniform
                # AP (4D strided inputs fall off the 2x_1p fast path)
                Hv = H[:].rearrange("p j g t -> p (j g) t")
                nc.vector.reduce_max(dst, Hv[:, :, 0:h], axis=AX.X)

            # ---- finale: ship the small collections; the host takes the
            # exact logs and w-weighted means (O(B1*B2*(S1+S2)) scalar
            # work, ~0.003% of the device FLOPs).
            lnS = coll_pool.tile([128, 512], F32, tag="lnS")

            def emit_finale_m2(h):
                """PSUM->SBUF copy + DMA for the h-th half-bank of SumE
                (needs only that half's Sigma chain, not the reduces)."""
                p0, p1 = 64 * h, 64 * (h + 1)
                nc.vector.tensor_copy(lnS[p0:p1, :], sumE[h][:])
                nc.sync.dma_start(sumEo.ap()[p0:p1, :], lnS[p0:p1, :])

            # software pipeline: PE mms for unit u, then E-consumers for
            # two lagged units at once (fewer DR<->onehot stationary
            # transitions on the PE).
            pending = []

            def drain_pending(n):
                while pending and len(pending) > n:
                    pu, pE = pending.pop(0)
                    emit_sum(pu, pE)
                    emit_red(pu, pE)
                    if pu == 31:
                        emit_finale_m2(0)

            for u in range(NCH * 4):
                S = emit_mm(u)
                E = emit_exp(u, S)
                pending.append((u, E))
                if u % 2 and len(pending) > DEFER:
                    drain_pending(DEFER - 1)
            # final drain: all Sigma-mms first so m2's half-1 chain closes
            # early, then the remaining maxE reduces, then m1.
            for pu, pE in pending:
                emit_sum(pu, pE)
            emit_finale_m2(1)
            for pu, pE in pending:
                emit_red(pu, pE)
            pending = []
            nc.sync.dma_start(maxEo.ap(), sim1st[:])
    nc.finalize()
    return nc


def _prep(x1, mask1, x2, mask2):
    """Host-side marshaling: normalize, mask-zero, sort b, quantize."""
    x1 = np.asarray(x1, dtype=np.float32)
    x2 = np.asarray(x2, dtype=np.float32)
    m1 = np.asarray(mask1).astype(bool)
    m2 = np.asarray(mask2).astype(bool)

    EPS = 1e-8
    n1 = np.sqrt((x1 * x1).sum(-1, keepdims=True))
    n2 = np.sqrt((x2 * x2).sum(-1, keepdims=True))
    x1n = (x1 / np.maximum(n1, EPS)) * QSCALE
    x2n = (x2 / np.maximum(n2, EPS)) * QSCALE
    x1n[~m1] = 0.0
    x2n[~m2] = 0.0

    len1 = m1.sum(axis=1).astype(np.int64)
    len2 = m2.sum(axis=1).astype(np.int64)
    ext2 = np.where(m2.any(1), S2 - np.argmax(m2[:, ::-1], axis=1), 1)
    b_order = np.argsort(-ext2, kind="stable")

    def _ev(v):
        v = int(max(v, 4))
        return (v + 3) // 4 * 4   # /2-able and 4B-aligned halves (2x_1p)

    L2G = tuple(_ev(ext2[b_order[4 * c]]) for c in range(NCH))

    w1 = m1.astype(np.float32) * (0.5 / np.maximum(len1, 1))[:, None]
    w2 = m2.astype(np.float32) * (0.5 / np.maximum(len2, 1))[:, None]
    w2s = w2[b_order]                                 # [64 sorted b, 128 t]

    # pack x2 columns: chunk c's 4 b's back-to-back at stride L2G[c]
    x2s = x2n[b_order]                                # [64, 128, 512]
    Wp = 4 * sum(L2G)
    x2pk = np.zeros((Wp, D), np.float32)
    off = 0
    for c in range(NCH):
        l = L2G[c]
        for g in range(4):
            x2pk[off : off + min(l, S2)] = x2s[4 * c + g, :l]
            off += l
    x2T = np.ascontiguousarray(x2pk.T).astype(E4NP)

    in_maps = []
    for k in range(NCORES):
        rows = slice(k * A, (k + 1) * A)
        x1T = np.ascontiguousarray(
            x1n[rows].reshape(X1W, D).T
        ).astype(E4NP)
        consts = np.zeros((128, 4), np.float32)
        consts[:, 0] = 1.0
        consts[:, 1] = -BETA * C0
        in_maps.append(
            {
                "x1p": x1T,
                "x2p": x2T,
                "consts": consts,
            }
        )
    return in_maps, b_order, L2G, (w1, w2s)


def kernel(x1, mask1, x2, mask2):
    in_maps, b_order, key, (w1, w2s) = _prep(x1, mask1, x2, mask2)
    if _CACHE.get("key") != key:
        _CACHE["nc"] = _build(key)
        _CACHE["key"] = key
    nc = _CACHE["nc"]
    res = run_bass_kernel_spmd(nc, in_maps, list(range(NCORES)))
    # host finale: exact logs + w-weighted means of the shipped
    # collections (both w tables already carry the 1/2 factor)
    w2v = w2s.reshape(NCH, 4, S2)                     # [c, g, t]
    outp = np.zeros((B1, B2), dtype=np.float32)
    for k in range(NCORES):
        maxE = res.results[k]["maxEo"].reshape(128, A, B2).astype(np.float32)
        sumE = res.results[k]["sumEo"]                # [128 (c,a), 512]
        lnm = np.log(np.maximum(maxE, 1e-38))         # [s, a, sorted b]
        m1v = np.einsum("sab,as->ab", lnm, w1[k * A : (k + 1) * A])
        lnS = np.log(np.maximum(sumE, 1e-45)).reshape(NCH, A, 4, S2)
        M2 = np.einsum("cagt,cgt->acg", lnS, w2v).reshape(A, B2)
        vals = (m1v + M2) / BETA + C0
        outp[np.ix_(range(k * A, (k + 1) * A), b_order)] = vals
    return np.ascontiguousarray(outp)
